# revision 18
# baseline (speedup 1.0000x reference)
"""Trainium2 Bass kernel for CRFIntegrationModule.

Math: for each pixel the reference accumulates confidence-weighted depth
estimates from up to 16 same-semantic neighbors in 4 directions (L/R/U/D),
with multiplicative path weights exp(sum of log-gradients), then blends.

Reformulation (validated vs reference in numpy):
  With S = (msk==1 ? sem : -1), Q = (msk==1)*exp(-min(var,5)), q = Q*dep,
  b[n] = [S[n-1]==S[n]], the LEFT-direction accumulators satisfy segmented
  linear recurrences along the row:
    A[n] = b[n]*e^{g[n-1]}*(q[n-1] + A[n-1])      (depth sum, unwindowed)
    B[n] = b[n]*(Q[n-1] + B[n-1])                 (conf sum,  unwindowed)
  and the 16-neighbor window is recovered by subtracting the tail:
    W[n]  = A[n] - gate[n]*e^{C[n-1]-C[n-17]}*A[n-16]
    Wc[n] = B[n] - gate[n]*B[n-16]
  where C = cumsum(g), gate[n] = [NB[n]==NB[n-16]], NB = cumsum([S[n-1]!=S[n]]).
  These map to DVE tensor_tensor_scan ops. RIGHT uses the mirrored recurrence
  (negative-stride scan). U/D are identical along columns, computed in a
  PE-transposed layout (7 column chunks x 168 rows incl. 20-row halos).

Sharding: pure data parallel, 8 cores = 4 images x 2 row-halves (128 rows).
Each core loads its own 20 halo rows; no cross-device communication.
"""
import sys
import numpy as np

sys.path.insert(0, "/opt/trn_rl_repo")

BZ, H, W = 4, 256, 832
HB = 128              # rows per core
PAD = 20              # horizontal pad (window reads reach 17 back)
WP = W + 2 * PAD      # 872
HALO = 20             # vertical halo rows each side
HH = 64               # halo pack: top at partitions 0..19, bottom at 32..51
HB0, HB1 = 32, 52     # bottom-halo partition range (legal matmul base)
CH = HALO + HB + HALO  # 168 rows per transposed chunk
NCH = 7               # 832 = 6*128 + 64 column chunks
VW = NCH * CH         # 1176
CW_FULL = NCH * HB    # 896 (center width in transposed space)
CWS = [128] * 6 + [64]
CLIPVAR = 5.0
LAM = 0.05

_prog = None


def _build(stage=4):
    import concourse.tile as tile
    import concourse.mybir as mybir
    from concourse import bacc, masks
    from contextlib import ExitStack

    Alu = mybir.AluOpType
    Act = mybir.ActivationFunctionType
    f32 = mybir.dt.float32
    i32 = mybir.dt.int32

    nc = bacc.Bacc("TRN2", target_bir_lowering=False, debug=False)

    d_sem = nc.dram_tensor("sem", [HB, W], i32, kind="ExternalInput").ap()
    d_msk = nc.dram_tensor("msk", [HB, W], i32, kind="ExternalInput").ap()
    d_var = nc.dram_tensor("var", [HB, W], f32, kind="ExternalInput").ap()
    d_dep = nc.dram_tensor("dep", [HB, W], f32, kind="ExternalInput").ap()
    d_dpi = nc.dram_tensor("dpi", [HB, W], f32, kind="ExternalInput").ap()
    d_g0 = nc.dram_tensor("g0", [HB, W], f32, kind="ExternalInput").ap()
    d_g1 = nc.dram_tensor("g1", [HB, W], f32, kind="ExternalInput").ap()
    d_hsem = nc.dram_tensor("hsem", [HH, W], i32, kind="ExternalInput").ap()
    d_hmsk = nc.dram_tensor("hmsk", [HH, W], i32, kind="ExternalInput").ap()
    d_hvar = nc.dram_tensor("hvar", [HH, W], f32, kind="ExternalInput").ap()
    d_hdep = nc.dram_tensor("hdep", [HH, W], f32, kind="ExternalInput").ap()
    d_hg1 = nc.dram_tensor("hg1", [HH, W], f32, kind="ExternalInput").ap()
    d_out = nc.dram_tensor("out", [HB, W], f32, kind="ExternalOutput").ap()

    CS = slice(PAD, PAD + W)

    with tile.TileContext(nc) as tc, ExitStack() as ctx:
        pool = ctx.enter_context(tc.tile_pool(name="pool", bufs=1))
        psum = ctx.enter_context(tc.tile_pool(name="psum", bufs=1, space="PSUM"))

        # ---------- loads ----------
        semi = pool.tile([HB, W], i32)
        mski = pool.tile([HB, W], i32)
        var = pool.tile([HB, W], f32)
        dep = pool.tile([HB, W], f32)
        dpi = pool.tile([HB, W], f32)
        g1m = pool.tile([HB, W], f32)
        nc.sync.dma_start(semi[:], d_sem)
        nc.sync.dma_start(mski[:], d_msk)
        nc.sync.dma_start(var[:], d_var)
        nc.sync.dma_start(dep[:], d_dep)
        nc.sync.dma_start(dpi[:], d_dpi)
        nc.sync.dma_start(g1m[:], d_g1)
        g0h = pool.tile([HB, WP], f32)
        nc.gpsimd.memset(g0h[:], 0.0)
        nc.sync.dma_start(g0h[:, CS], d_g0)
        hsemi = pool.tile([HH, W], i32)
        hmski = pool.tile([HH, W], i32)
        hvar = pool.tile([HH, W], f32)
        hdep = pool.tile([HH, W], f32)
        hg1 = pool.tile([HH, W], f32)
        nc.sync.dma_start(hsemi[:], d_hsem)
        nc.sync.dma_start(hmski[:], d_hmsk)
        nc.sync.dma_start(hvar[:], d_hvar)
        nc.sync.dma_start(hdep[:], d_hdep)
        nc.sync.dma_start(hg1[:], d_hg1)

        # ---------- precompute S, Q, q (main + halo) ----------
        S = pool.tile([HB, WP], f32)
        Q = pool.tile([HB, WP], f32)
        q = pool.tile([HB, WP], f32)
        nc.gpsimd.memset(S[:], -1.0)
        nc.gpsimd.memset(Q[:], 0.0)
        nc.gpsimd.memset(q[:], 0.0)

        m1f = pool.tile([HB, W], f32)
        semf = pool.tile([HB, W], f32, tag="pre", bufs=2)
        nc.vector.tensor_copy(semf[:], semi[:])
        nc.vector.tensor_copy(m1f[:], mski[:])
        st = pool.tile([HB, W], f32, tag="pre", bufs=2)
        nc.vector.scalar_tensor_tensor(st[:], semf[:], 1.0, m1f[:],
                                       Alu.add, Alu.mult)
        nc.vector.tensor_scalar_sub(S[:, CS], st[:], 1.0)
        vm = pool.tile([HB, W], f32, tag="pre", bufs=2)
        nc.vector.tensor_scalar(vm[:], var[:], CLIPVAR, -1.0, Alu.min, Alu.mult)
        Etmp = pool.tile([HB, W], f32, tag="pre", bufs=2)
        nc.scalar.activation(Etmp[:], vm[:], Act.Exp)
        nc.vector.tensor_tensor(Q[:, CS], Etmp[:], m1f[:], Alu.mult)
        nc.vector.tensor_tensor(q[:, CS], Q[:, CS], dep[:], Alu.mult)

        hS = pool.tile([HH, W], f32)
        hQ = pool.tile([HH, W], f32)
        hq = pool.tile([HH, W], f32)
        hm1f = pool.tile([HH, W], f32)
        hsemf = pool.tile([HH, W], f32, tag="hpre", bufs=2)
        nc.vector.tensor_copy(hsemf[:], hsemi[:])
        nc.vector.tensor_copy(hm1f[:], hmski[:])
        hst = pool.tile([HH, W], f32, tag="hpre", bufs=2)
        nc.vector.scalar_tensor_tensor(hst[:], hsemf[:], 1.0, hm1f[:],
                                       Alu.add, Alu.mult)
        nc.vector.tensor_scalar_sub(hS[:], hst[:], 1.0)
        hvm = pool.tile([HH, W], f32, tag="hpre", bufs=2)
        nc.vector.tensor_scalar(hvm[:], hvar[:], CLIPVAR, -1.0, Alu.min, Alu.mult)
        hE = pool.tile([HH, W], f32, tag="hpre", bufs=2)
        nc.scalar.activation(hE[:], hvm[:], Act.Exp)
        nc.vector.tensor_tensor(hQ[:], hE[:], hm1f[:], Alu.mult)
        nc.vector.tensor_tensor(hq[:], hQ[:], hdep[:], Alu.mult)

        # ---------- transposed (vertical) planes ----------
        ident = pool.tile([HB, HB], f32)
        masks.make_identity(nc, ident[:])

        Sv = pool.tile([HB, VW], f32)
        Qv = pool.tile([HB, VW], f32)
        qv = pool.tile([HB, VW], f32)
        gv = pool.tile([HB, VW], f32)
        nc.gpsimd.memset(Sv[:], -1.0)
        nc.gpsimd.memset(Qv[:], 0.0)
        nc.gpsimd.memset(qv[:], 0.0)
        nc.gpsimd.memset(gv[:], 0.0)

        plane_srcs = [
            (S, hS, Sv, True),
            (Q, hQ, Qv, True),
            (q, hq, qv, True),
            (g1m, hg1, gv, False),
        ]
        for c in range(NCH if stage >= 2 else 0):
            cw = CWS[c]
            c0 = c * 128
            for (main, halo, dst, padded) in plane_srcs:
                mv = main[:, PAD + c0:PAD + c0 + cw] if padded else main[:, c0:c0 + cw]
                pt1 = psum.tile([HB, HB], f32, tag="ptin1", bufs=2)
                pt2 = psum.tile([HB, HALO], f32, tag="ptin2", bufs=2)
                pt3 = psum.tile([HB, HALO], f32, tag="ptin3", bufs=2)
                nc.tensor.transpose(pt1[:cw, :], mv, ident[:])
                nc.tensor.transpose(pt2[:cw, :], halo[0:HALO, c0:c0 + cw],
                                    ident[0:HALO, 0:HALO])
                nc.tensor.transpose(pt3[:cw, :], halo[HB0:HB1, c0:c0 + cw],
                                    ident[HB0:HB1, HB0:HB1])
                dslot = dst[:cw, c * CH:(c + 1) * CH]
                nc.scalar.copy(dslot[:, HALO:HALO + HB], pt1[:cw, :])
                nc.scalar.copy(dslot[:, 0:HALO], pt2[:cw, :])
                nc.scalar.copy(dslot[:, HALO + HB:CH], pt3[:cw, :])

        # ---------- direction passes ----------
        def directions(Sx, Qx, qx, gx, WD, dsum_add, csum_add):
            """Emit fwd+rev direction pair over [HB, WD] planes."""
            is_v = WD == VW
            CWIDTH = CW_FULL if is_v else W

            def c3(x):
                return x.rearrange("p (c f) -> p c f", c=NCH) if is_v else x

            def center(x, off):
                if not is_v:
                    return x[:, PAD + off:PAD + off + W]
                v = x.rearrange("p (c f) -> p c f", c=NCH)
                return v[:, :, HALO + off:HALO + off + HB]

            b = pool.tile([HB, VW], f32, tag="dir_b")
            nb = pool.tile([HB, VW], f32, tag="dir_be")
            nc.vector.tensor_tensor(b[:, 1:WD], Sx[:, :WD - 1], Sx[:, 1:WD],
                                    Alu.is_equal)
            nc.vector.memset(b[:, 0:1], 0.0)
            if is_v:
                for c in range(1, NCH):
                    nc.vector.memset(b[:, c * CH:c * CH + 1], 0.0)
            # nb = 1 - b (1-input: line rate on gpsimd)
            nc.gpsimd.tensor_scalar(nb[:, :WD], b[:, :WD], 1.0, -1.0,
                                    Alu.subtract, Alu.mult)
            NBt = pool.tile([HB, VW], f32, tag="dir_NB")
            nc.vector.tensor_tensor_scan(NBt[:, :WD], nb[:, :WD], nb[:, :WD],
                                         0.0, Alu.add, Alu.bypass)
            Ct = pool.tile([HB, VW], f32, tag="dir_C")
            nc.vector.tensor_tensor_scan(Ct[:, :WD], gx[:, :WD], gx[:, :WD],
                                         0.0, Alu.add, Alu.bypass)

            for rev in (False, True):
                eg = pool.tile([HB, VW], f32, tag="dir_eg")
                be = pool.tile([HB, VW], f32, tag="dir_be")
                A = pool.tile([HB, VW], f32, tag="dir_A")
                B = pool.tile([HB, VW], f32, tag="dir_B")
                if not rev:
                    nc.scalar.activation(eg[:, :WD], gx[:, :WD], Act.Exp)
                    nc.vector.tensor_tensor(be[:, 1:WD], b[:, 1:WD],
                                            eg[:, :WD - 1], Alu.mult)
                    nc.vector.memset(A[:, 0:1], 0.0)
                    nc.vector.memset(B[:, 0:1], 0.0)
                    nc.vector.tensor_tensor_scan(
                        A[:, 1:WD], qx[:, :WD - 1], be[:, 1:WD],
                        0.0, Alu.add, Alu.mult)
                    nc.vector.tensor_tensor_scan(
                        B[:, 1:WD], Qx[:, :WD - 1], b[:, 1:WD],
                        0.0, Alu.add, Alu.mult)
                    g_o, t_o = 0, -16          # gate: NB[n] vs NB[n-16]
                    r_a, r_b = -1, -17         # ratio: exp(C[n-1]-C[n-17])
                else:
                    nc.scalar.activation(eg[:, :WD], gx[:, :WD], Act.Exp, scale=-1.0)
                    nc.vector.tensor_tensor(be[:, 0:WD - 1], b[:, 1:WD],
                                            eg[:, :WD - 1], Alu.mult)
                    nc.vector.memset(A[:, WD - 1:WD], 0.0)
                    nc.vector.memset(B[:, WD - 1:WD], 0.0)
                    nc.vector.tensor_tensor_scan(
                        A[:, 0:WD - 1][:, ::-1], qx[:, 1:WD][:, ::-1],
                        be[:, 0:WD - 1][:, ::-1], 0.0, Alu.add, Alu.mult)
                    nc.vector.tensor_tensor_scan(
                        B[:, 0:WD - 1][:, ::-1], Qx[:, 1:WD][:, ::-1],
                        b[:, 1:WD][:, ::-1], 0.0, Alu.add, Alu.mult)
                    g_o, t_o = 16, 16          # gate: NB[n+16] vs NB[n]
                    r_a, r_b = -1, 15          # ratio: exp(C[n-1]-C[n+15])

                gate = pool.tile([HB, CW_FULL], f32, tag="dir_gate")
                ratio = pool.tile([HB, CW_FULL], f32, tag="dir_ratio")
                TA = pool.tile([HB, CW_FULL], f32, tag="dir_TA")
                TB = pool.tile([HB, CW_FULL], f32, tag="dir_TB")
                gv_ = c3(gate[:, :CWIDTH])
                rv_ = c3(ratio[:, :CWIDTH])
                tv_ = c3(TA[:, :CWIDTH])
                tb_ = c3(TB[:, :CWIDTH])
                nc.vector.tensor_tensor(gv_, center(NBt, g_o), center(NBt, g_o - 16),
                                        Alu.is_equal)
                # ratio = exp(C[a] - C[b]) : one TT + one ACT (exact window sum)
                nc.vector.tensor_tensor(rv_, center(Ct, r_a), center(Ct, r_b),
                                        Alu.subtract)
                nc.scalar.activation(rv_, rv_, Act.Exp)
                nc.vector.tensor_tensor(tv_, center(A, t_o), rv_, Alu.mult)
                nc.vector.tensor_tensor(tv_, tv_, gv_, Alu.mult)
                dsum_add('+', center(A, 0))
                dsum_add('-', tv_)
                nc.vector.tensor_tensor(tb_, center(B, t_o), gv_, Alu.mult)
                csum_add('+', center(B, 0))
                csum_add('-', tb_)

        # ---------- accumulators ----------
        dsum = pool.tile([HB, W], f32)
        csum = pool.tile([HB, W], f32)
        nc.scalar.copy(dsum[:], q[:, CS])
        nc.scalar.copy(csum[:], Q[:, CS])

        def h_acc(acc):
            def add(kind, ap):
                nc.gpsimd.tensor_tensor(
                    acc[:], acc[:], ap, Alu.add if kind == '+' else Alu.subtract)
            return add

        if stage >= 1:
            directions(S, Q, q, g0h, WP, h_acc(dsum), h_acc(csum))

        Wv = pool.tile([HB, CW_FULL], f32)
        Wcv = pool.tile([HB, CW_FULL], f32)
        vstate = {"d": 0, "c": 0}

        def v_acc(acc, key):
            t = acc.rearrange("p (c f) -> p c f", c=NCH)

            def add(kind, ap):
                if vstate[key] == 0:
                    nc.gpsimd.tensor_copy(t[:], ap)
                else:
                    nc.gpsimd.tensor_tensor(
                        t[:], t[:], ap, Alu.add if kind == '+' else Alu.subtract)
                vstate[key] += 1
            return add

        if stage >= 3:
            directions(Sv, Qv, qv, gv, VW, v_acc(Wv, "d"), v_acc(Wcv, "c"))

        for c in range(NCH if stage >= 4 else 0):
            cw = CWS[c]
            c0 = c * 128
            for (src, acc) in ((Wv, dsum), (Wcv, csum)):
                pt = psum.tile([HB, HB], f32, tag="ptout", bufs=2)
                nc.tensor.transpose(pt[:HB, :cw], src[:cw, c * HB:c * HB + HB],
                                    ident[0:cw, 0:cw])
                nc.vector.tensor_tensor(acc[:, c0:c0 + cw], acc[:, c0:c0 + cw],
                                        pt[:HB, :cw], Alu.add)

        # ---------- final blend ----------
        # out = where(lat > 0, LAM*dpi + (1-LAM)*lat, dpi); lat >= 0 always,
        # so "lat nonzero" == "lat > 0" and lat itself serves as the mask.
        dpi05 = pool.tile([HB, W], f32)
        nc.scalar.mul(dpi05[:], dpi[:], LAM)
        outt = pool.tile([HB, W], f32)
        nc.scalar.copy(outt[:], dpi[:])
        mx = pool.tile([HB, W], f32, tag="fin", bufs=3)
        nc.vector.tensor_scalar_max(mx[:], csum[:], 1e-12)
        rcp = pool.tile([HB, W], f32, tag="fin", bufs=3)
        scr = pool.tile([HB, W], f32, tag="fin", bufs=3)
        nc.vector.reciprocal_approx_accurate(rcp[:], mx[:], scr[:])
        lat = pool.tile([HB, W], f32, tag="fin", bufs=3)
        nc.vector.tensor_tensor(lat[:], dsum[:], rcp[:], Alu.mult)
        cp = pool.tile([HB, W], f32, tag="fin", bufs=3)
        nc.vector.scalar_tensor_tensor(cp[:], lat[:], 1.0 - LAM, dpi05[:],
                                       Alu.mult, Alu.add)
        nc.vector.copy_predicated(outt[:], lat[:].bitcast(i32), cp[:])
        nc.sync.dma_start(d_out, outt[:])

    nc.compile()
    return nc


def _get_prog():
    global _prog
    if _prog is None:
        _prog = _build()
    return _prog


def _core_maps(pred_log, semantics, mask, variance, dep_cur, dep_orig):
    maps = []
    for c in range(8):
        b, r = c // 2, c % 2
        r0 = r * HB
        sem = semantics[b, 0]
        msk = mask[b, 0]
        var = variance[b, 0]
        dep = dep_cur[b, 0]
        g1 = pred_log[b, 1]
        hsem = np.full((HH, W), -1, np.int32)
        hmsk = np.zeros((HH, W), np.int32)
        hvar = np.zeros((HH, W), np.float32)
        hdep = np.zeros((HH, W), np.float32)
        hg1 = np.zeros((HH, W), np.float32)
        if r0 - HALO >= 0:
            sl = slice(r0 - HALO, r0)
            hsem[:HALO] = sem[sl]
            hmsk[:HALO] = msk[sl]
            hvar[:HALO] = var[sl]
            hdep[:HALO] = dep[sl]
            hg1[:HALO] = g1[sl]
        if r0 + HB + HALO <= H:
            sl = slice(r0 + HB, r0 + HB + HALO)
            hsem[HB0:HB1] = sem[sl]
            hmsk[HB0:HB1] = msk[sl]
            hvar[HB0:HB1] = var[sl]
            hdep[HB0:HB1] = dep[sl]
            hg1[HB0:HB1] = g1[sl]
        rs = slice(r0, r0 + HB)
        maps.append({
            "sem": np.ascontiguousarray(sem[rs], np.int32),
            "msk": np.ascontiguousarray(msk[rs], np.int32),
            "var": np.ascontiguousarray(var[rs], np.float32),
            "dep": np.ascontiguousarray(dep[rs], np.float32),
            "dpi": np.ascontiguousarray(dep_orig[b, 0][rs], np.float32),
            "g0": np.ascontiguousarray(pred_log[b, 0][rs], np.float32),
            "g1": np.ascontiguousarray(g1[rs], np.float32),
            "hsem": hsem, "hmsk": hmsk, "hvar": hvar, "hdep": hdep, "hg1": hg1,
        })
    return maps


PROFILE = False
LAST_RESULT = None


def _run_once(pred_log, semantics, mask, variance, dep_cur, dep_orig):
    global LAST_RESULT
    from concourse.bass_utils import run_bass_kernel_spmd

    nc = _get_prog()
    in_maps = _core_maps(pred_log, semantics, mask, variance, dep_cur, dep_orig)
    res = run_bass_kernel_spmd(nc, in_maps, core_ids=list(range(8)),
                               trace=PROFILE)
    LAST_RESULT = res
    out = np.empty((BZ, 1, H, W), np.float32)
    for c in range(8):
        b, r = c // 2, c % 2
        out[b, 0, r * HB:(r + 1) * HB] = res.results[c]["out"]
    return out


def kernel(pred_log, semantics, mask, variance, depthin, times=1):
    pred_log = np.asarray(pred_log, np.float32)
    semantics = np.asarray(semantics)
    mask = np.asarray(mask)
    variance = np.asarray(variance, np.float32)
    depthin = np.asarray(depthin, np.float32).reshape(BZ, 1, H, W)
    t = int(np.asarray(times))
    depthout = depthin
    for _ in range(t):
        depthout = _run_once(pred_log, semantics, mask, variance,
                             depthout, depthin)
    if t == 0:
        depthout = depthin.copy()
    return depthout


# revision 19
# speedup vs baseline: 1.4022x; 1.4022x over previous
"""Trainium2 Bass kernel for CRFIntegrationModule.

Math: for each pixel the reference accumulates confidence-weighted depth
estimates from up to 16 same-semantic neighbors in 4 directions (L/R/U/D),
with multiplicative path weights exp(sum of log-gradients), then blends.

Reformulation (validated vs reference in numpy):
  With S = (msk==1 ? sem : -1), Q = (msk==1)*exp(-min(var,5)), q = Q*dep,
  b[n] = [S[n-1]==S[n]], the LEFT-direction accumulators satisfy segmented
  linear recurrences along the row:
    A[n] = b[n]*e^{g[n-1]}*(q[n-1] + A[n-1])      (depth sum, unwindowed)
    B[n] = b[n]*(Q[n-1] + B[n-1])                 (conf sum,  unwindowed)
  and the 16-neighbor window is recovered by subtracting the tail:
    W[n]  = A[n] - gate[n]*e^{C[n-1]-C[n-17]}*A[n-16]
    Wc[n] = B[n] - gate[n]*B[n-16]
  where C = cumsum(g), gate[n] = [NB[n]==NB[n-16]], NB = cumsum([S[n-1]!=S[n]]).
  These map to DVE tensor_tensor_scan ops. RIGHT uses the mirrored recurrence
  (negative-stride scan). U/D are identical along columns, computed in a
  PE-transposed layout (7 column chunks x 168 rows incl. 20-row halos).

Sharding: pure data parallel, 8 cores = 4 images x 2 row-halves (128 rows).
Each core loads its own 20 halo rows; no cross-device communication.
"""
import sys
import numpy as np

sys.path.insert(0, "/opt/trn_rl_repo")

BZ, H, W = 4, 256, 832
HB = 128              # rows per core
PAD = 20              # horizontal pad (window reads reach 17 back)
WP = W + 2 * PAD      # 872
HALO = 20             # vertical halo rows each side
HH = 64               # halo pack: top at partitions 0..19, bottom at 32..51
HB0, HB1 = 32, 52     # bottom-halo partition range (legal matmul base)
CH = HALO + HB + HALO  # 168 rows per transposed chunk
NCH = 7               # 832 = 6*128 + 64 column chunks
VW = NCH * CH         # 1176
CW_FULL = NCH * HB    # 896 (center width in transposed space)
CWS = [128] * 6 + [64]
CLIPVAR = 5.0
LAM = 0.05

_prog = None


def _build(stage=4):
    import concourse.tile as tile
    import concourse.mybir as mybir
    from concourse import bacc, masks
    from contextlib import ExitStack

    Alu = mybir.AluOpType
    Act = mybir.ActivationFunctionType
    f32 = mybir.dt.float32
    i32 = mybir.dt.int32

    nc = bacc.Bacc("TRN2", target_bir_lowering=False, debug=False)

    d_sem = nc.dram_tensor("sem", [HB, W], i32, kind="ExternalInput").ap()
    d_msk = nc.dram_tensor("msk", [HB, W], i32, kind="ExternalInput").ap()
    d_var = nc.dram_tensor("var", [HB, W], f32, kind="ExternalInput").ap()
    d_dep = nc.dram_tensor("dep", [HB, W], f32, kind="ExternalInput").ap()
    d_dpi = nc.dram_tensor("dpi", [HB, W], f32, kind="ExternalInput").ap()
    d_g0 = nc.dram_tensor("g0", [HB, W], f32, kind="ExternalInput").ap()
    d_g1 = nc.dram_tensor("g1", [HB, W], f32, kind="ExternalInput").ap()
    d_hsem = nc.dram_tensor("hsem", [HH, W], i32, kind="ExternalInput").ap()
    d_hmsk = nc.dram_tensor("hmsk", [HH, W], i32, kind="ExternalInput").ap()
    d_hvar = nc.dram_tensor("hvar", [HH, W], f32, kind="ExternalInput").ap()
    d_hdep = nc.dram_tensor("hdep", [HH, W], f32, kind="ExternalInput").ap()
    d_hg1 = nc.dram_tensor("hg1", [HH, W], f32, kind="ExternalInput").ap()
    d_out = nc.dram_tensor("out", [HB, W], f32, kind="ExternalOutput").ap()

    CS = slice(PAD, PAD + W)

    with tile.TileContext(nc) as tc, ExitStack() as ctx:
        pool = ctx.enter_context(tc.tile_pool(name="pool", bufs=1))
        psum = ctx.enter_context(tc.tile_pool(name="psum", bufs=1, space="PSUM"))

        # ---------- loads ----------
        semi = pool.tile([HB, W], i32)
        mski = pool.tile([HB, W], i32)
        var = pool.tile([HB, W], f32)
        dep = pool.tile([HB, W], f32)
        dpi = pool.tile([HB, W], f32)
        g1m = pool.tile([HB, W], f32)
        nc.sync.dma_start(semi[:], d_sem)
        nc.sync.dma_start(mski[:], d_msk)
        nc.sync.dma_start(var[:], d_var)
        nc.sync.dma_start(dep[:], d_dep)
        nc.sync.dma_start(dpi[:], d_dpi)
        nc.sync.dma_start(g1m[:], d_g1)
        g0h = pool.tile([HB, WP], f32)
        nc.gpsimd.memset(g0h[:], 0.0)
        nc.sync.dma_start(g0h[:, CS], d_g0)
        hsemi = pool.tile([HH, W], i32)
        hmski = pool.tile([HH, W], i32)
        hvar = pool.tile([HH, W], f32)
        hdep = pool.tile([HH, W], f32)
        hg1 = pool.tile([HH, W], f32)
        nc.sync.dma_start(hsemi[:], d_hsem)
        nc.sync.dma_start(hmski[:], d_hmsk)
        nc.sync.dma_start(hvar[:], d_hvar)
        nc.sync.dma_start(hdep[:], d_hdep)
        nc.sync.dma_start(hg1[:], d_hg1)

        # ---------- precompute S, Q, q (main + halo) ----------
        S = pool.tile([HB, WP], f32)
        Q = pool.tile([HB, WP], f32)
        q = pool.tile([HB, WP], f32)
        nc.gpsimd.memset(S[:], -1.0)
        nc.gpsimd.memset(Q[:], 0.0)
        nc.gpsimd.memset(q[:], 0.0)

        m1f = pool.tile([HB, W], f32)
        semf = pool.tile([HB, W], f32, tag="pre", bufs=2)
        nc.vector.tensor_copy(semf[:], semi[:])
        nc.vector.tensor_copy(m1f[:], mski[:])
        st = pool.tile([HB, W], f32, tag="pre", bufs=2)
        nc.vector.scalar_tensor_tensor(st[:], semf[:], 1.0, m1f[:],
                                       Alu.add, Alu.mult)
        nc.vector.tensor_scalar_sub(S[:, CS], st[:], 1.0)
        vm = pool.tile([HB, W], f32, tag="pre", bufs=2)
        nc.vector.tensor_scalar(vm[:], var[:], CLIPVAR, -1.0, Alu.min, Alu.mult)
        Etmp = pool.tile([HB, W], f32, tag="pre", bufs=2)
        nc.scalar.activation(Etmp[:], vm[:], Act.Exp)
        nc.vector.tensor_tensor(Q[:, CS], Etmp[:], m1f[:], Alu.mult)
        nc.vector.tensor_tensor(q[:, CS], Q[:, CS], dep[:], Alu.mult)

        hS = pool.tile([HH, W], f32)
        hQ = pool.tile([HH, W], f32)
        hq = pool.tile([HH, W], f32)
        hm1f = pool.tile([HH, W], f32)
        hsemf = pool.tile([HH, W], f32, tag="hpre", bufs=2)
        nc.vector.tensor_copy(hsemf[:], hsemi[:])
        nc.vector.tensor_copy(hm1f[:], hmski[:])
        hst = pool.tile([HH, W], f32, tag="hpre", bufs=2)
        nc.vector.scalar_tensor_tensor(hst[:], hsemf[:], 1.0, hm1f[:],
                                       Alu.add, Alu.mult)
        nc.vector.tensor_scalar_sub(hS[:], hst[:], 1.0)
        hvm = pool.tile([HH, W], f32, tag="hpre", bufs=2)
        nc.vector.tensor_scalar(hvm[:], hvar[:], CLIPVAR, -1.0, Alu.min, Alu.mult)
        hE = pool.tile([HH, W], f32, tag="hpre", bufs=2)
        nc.scalar.activation(hE[:], hvm[:], Act.Exp)
        nc.vector.tensor_tensor(hQ[:], hE[:], hm1f[:], Alu.mult)
        nc.vector.tensor_tensor(hq[:], hQ[:], hdep[:], Alu.mult)

        # ---------- transposed (vertical) planes ----------
        ident = pool.tile([HB, HB], f32)
        masks.make_identity(nc, ident[:])

        Sv = pool.tile([HB, VW], f32)
        Qv = pool.tile([HB, VW], f32)
        qv = pool.tile([HB, VW], f32)
        gv = pool.tile([HB, VW], f32)
        nc.gpsimd.memset(Sv[:], -1.0)
        nc.gpsimd.memset(Qv[:], 0.0)
        nc.gpsimd.memset(qv[:], 0.0)
        nc.gpsimd.memset(gv[:], 0.0)

        plane_srcs = [
            (S, hS, Sv, True),
            (Q, hQ, Qv, True),
            (q, hq, qv, True),
            (g1m, hg1, gv, False),
        ]
        for c in range(NCH if stage >= 2 else 0):
            cw = CWS[c]
            c0 = c * 128
            for (main, halo, dst, padded) in plane_srcs:
                mv = main[:, PAD + c0:PAD + c0 + cw] if padded else main[:, c0:c0 + cw]
                pt1 = psum.tile([HB, HB], f32, tag="ptin1", bufs=2)
                pt2 = psum.tile([HB, HALO], f32, tag="ptin2", bufs=2)
                pt3 = psum.tile([HB, HALO], f32, tag="ptin3", bufs=2)
                nc.tensor.transpose(pt1[:cw, :], mv, ident[:])
                nc.tensor.transpose(pt2[:cw, :], halo[0:HALO, c0:c0 + cw],
                                    ident[0:HALO, 0:HALO])
                nc.tensor.transpose(pt3[:cw, :], halo[HB0:HB1, c0:c0 + cw],
                                    ident[HB0:HB1, HB0:HB1])
                dslot = dst[:cw, c * CH:(c + 1) * CH]
                nc.scalar.copy(dslot[:, HALO:HALO + HB], pt1[:cw, :])
                nc.scalar.copy(dslot[:, 0:HALO], pt2[:cw, :])
                nc.scalar.copy(dslot[:, HALO + HB:CH], pt3[:cw, :])

        # ---------- direction passes ----------
        def directions(Sx, Qx, qx, gx, WD, dsum_add, csum_add):
            """Emit fwd+rev direction pair over [HB, WD] planes."""
            is_v = WD == VW
            CWIDTH = CW_FULL if is_v else W

            def c3(x):
                return x.rearrange("p (c f) -> p c f", c=NCH) if is_v else x

            def center(x, off):
                if not is_v:
                    return x[:, PAD + off:PAD + off + W]
                v = x.rearrange("p (c f) -> p c f", c=NCH)
                return v[:, :, HALO + off:HALO + off + HB]

            b = pool.tile([HB, VW], f32, tag="dir_b")
            nb = pool.tile([HB, VW], f32, tag="dir_be")
            nc.vector.tensor_tensor(b[:, 1:WD], Sx[:, :WD - 1], Sx[:, 1:WD],
                                    Alu.is_equal)
            nc.vector.memset(b[:, 0:1], 0.0)
            if is_v:
                for c in range(1, NCH):
                    nc.vector.memset(b[:, c * CH:c * CH + 1], 0.0)
            # nb = 1 - b
            nc.vector.tensor_scalar(nb[:, :WD], b[:, :WD], 1.0, -1.0,
                                    Alu.subtract, Alu.mult)
            NBt = pool.tile([HB, VW], f32, tag="dir_NB")
            nc.vector.tensor_tensor_scan(NBt[:, :WD], nb[:, :WD], nb[:, :WD],
                                         0.0, Alu.add, Alu.bypass)
            Ct = pool.tile([HB, VW], f32, tag="dir_C")
            nc.vector.tensor_tensor_scan(Ct[:, :WD], gx[:, :WD], gx[:, :WD],
                                         0.0, Alu.add, Alu.bypass)

            for rev in (False, True):
                eg = pool.tile([HB, VW], f32, tag="dir_eg")
                be = pool.tile([HB, VW], f32, tag="dir_be")
                A = pool.tile([HB, VW], f32, tag="dir_A")
                B = pool.tile([HB, VW], f32, tag="dir_B")
                if not rev:
                    nc.scalar.activation(eg[:, :WD], gx[:, :WD], Act.Exp)
                    nc.vector.tensor_tensor(be[:, 1:WD], b[:, 1:WD],
                                            eg[:, :WD - 1], Alu.mult)
                    nc.vector.memset(A[:, 0:1], 0.0)
                    nc.vector.memset(B[:, 0:1], 0.0)
                    nc.vector.tensor_tensor_scan(
                        A[:, 1:WD], qx[:, :WD - 1], be[:, 1:WD],
                        0.0, Alu.add, Alu.mult)
                    nc.vector.tensor_tensor_scan(
                        B[:, 1:WD], Qx[:, :WD - 1], b[:, 1:WD],
                        0.0, Alu.add, Alu.mult)
                    g_o, t_o = 0, -16          # gate: NB[n] vs NB[n-16]
                    r_a, r_b = -1, -17         # ratio: exp(C[n-1]-C[n-17])
                else:
                    nc.scalar.activation(eg[:, :WD], gx[:, :WD], Act.Exp, scale=-1.0)
                    nc.vector.tensor_tensor(be[:, 0:WD - 1], b[:, 1:WD],
                                            eg[:, :WD - 1], Alu.mult)
                    nc.vector.memset(A[:, WD - 1:WD], 0.0)
                    nc.vector.memset(B[:, WD - 1:WD], 0.0)
                    nc.vector.tensor_tensor_scan(
                        A[:, 0:WD - 1][:, ::-1], qx[:, 1:WD][:, ::-1],
                        be[:, 0:WD - 1][:, ::-1], 0.0, Alu.add, Alu.mult)
                    nc.vector.tensor_tensor_scan(
                        B[:, 0:WD - 1][:, ::-1], Qx[:, 1:WD][:, ::-1],
                        b[:, 1:WD][:, ::-1], 0.0, Alu.add, Alu.mult)
                    g_o, t_o = 16, 16          # gate: NB[n+16] vs NB[n]
                    r_a, r_b = -1, 15          # ratio: exp(C[n-1]-C[n+15])

                gate = pool.tile([HB, CW_FULL], f32, tag="dir_gate")
                ratio = pool.tile([HB, CW_FULL], f32, tag="dir_ratio")
                TA = pool.tile([HB, CW_FULL], f32, tag="dir_TA")
                TB = pool.tile([HB, CW_FULL], f32, tag="dir_TB")
                gv_ = c3(gate[:, :CWIDTH])
                rv_ = c3(ratio[:, :CWIDTH])
                tv_ = c3(TA[:, :CWIDTH])
                tb_ = c3(TB[:, :CWIDTH])
                nc.vector.tensor_tensor(gv_, center(NBt, g_o), center(NBt, g_o - 16),
                                        Alu.is_equal)
                # ratio = exp(C[a] - C[b]) : one TT + one ACT (exact window sum)
                nc.vector.tensor_tensor(rv_, center(Ct, r_a), center(Ct, r_b),
                                        Alu.subtract)
                nc.scalar.activation(rv_, rv_, Act.Exp)
                nc.vector.tensor_tensor(tv_, center(A, t_o), rv_, Alu.mult)
                nc.vector.tensor_tensor(tv_, tv_, gv_, Alu.mult)
                dsum_add('+', center(A, 0))
                dsum_add('-', tv_)
                nc.vector.tensor_tensor(tb_, center(B, t_o), gv_, Alu.mult)
                csum_add('+', center(B, 0))
                csum_add('-', tb_)

        # ---------- accumulators ----------
        dsum = pool.tile([HB, W], f32)
        csum = pool.tile([HB, W], f32)
        nc.scalar.copy(dsum[:], q[:, CS])
        nc.scalar.copy(csum[:], Q[:, CS])

        def h_acc(acc):
            def add(kind, ap):
                nc.vector.tensor_tensor(
                    acc[:], acc[:], ap, Alu.add if kind == '+' else Alu.subtract)
            return add

        if stage >= 1:
            directions(S, Q, q, g0h, WP, h_acc(dsum), h_acc(csum))

        Wv = pool.tile([HB, CW_FULL], f32)
        Wcv = pool.tile([HB, CW_FULL], f32)
        vstate = {"d": 0, "c": 0}

        def v_acc(acc, key):
            t = acc.rearrange("p (c f) -> p c f", c=NCH)

            def add(kind, ap):
                if vstate[key] == 0:
                    nc.vector.tensor_copy(t[:], ap)
                else:
                    nc.vector.tensor_tensor(
                        t[:], t[:], ap, Alu.add if kind == '+' else Alu.subtract)
                vstate[key] += 1
            return add

        if stage >= 3:
            directions(Sv, Qv, qv, gv, VW, v_acc(Wv, "d"), v_acc(Wcv, "c"))

        for c in range(NCH if stage >= 4 else 0):
            cw = CWS[c]
            c0 = c * 128
            for (src, acc) in ((Wv, dsum), (Wcv, csum)):
                pt = psum.tile([HB, HB], f32, tag="ptout", bufs=2)
                nc.tensor.transpose(pt[:HB, :cw], src[:cw, c * HB:c * HB + HB],
                                    ident[0:cw, 0:cw])
                nc.vector.tensor_tensor(acc[:, c0:c0 + cw], acc[:, c0:c0 + cw],
                                        pt[:HB, :cw], Alu.add)

        # ---------- final blend ----------
        # out = where(lat > 0, LAM*dpi + (1-LAM)*lat, dpi); lat >= 0 always,
        # so "lat nonzero" == "lat > 0" and lat itself serves as the mask.
        dpi05 = pool.tile([HB, W], f32)
        nc.scalar.mul(dpi05[:], dpi[:], LAM)
        outt = pool.tile([HB, W], f32)
        nc.scalar.copy(outt[:], dpi[:])
        mx = pool.tile([HB, W], f32, tag="fin", bufs=3)
        nc.vector.tensor_scalar_max(mx[:], csum[:], 1e-12)
        rcp = pool.tile([HB, W], f32, tag="fin", bufs=3)
        scr = pool.tile([HB, W], f32, tag="fin", bufs=3)
        nc.vector.reciprocal_approx_accurate(rcp[:], mx[:], scr[:])
        lat = pool.tile([HB, W], f32, tag="fin", bufs=3)
        nc.vector.tensor_tensor(lat[:], dsum[:], rcp[:], Alu.mult)
        cp = pool.tile([HB, W], f32, tag="fin", bufs=3)
        nc.vector.scalar_tensor_tensor(cp[:], lat[:], 1.0 - LAM, dpi05[:],
                                       Alu.mult, Alu.add)
        nc.vector.copy_predicated(outt[:], lat[:].bitcast(i32), cp[:])
        nc.sync.dma_start(d_out, outt[:])

    nc.compile()
    return nc


def _get_prog():
    global _prog
    if _prog is None:
        _prog = _build()
    return _prog


def _core_maps(pred_log, semantics, mask, variance, dep_cur, dep_orig):
    maps = []
    for c in range(8):
        b, r = c // 2, c % 2
        r0 = r * HB
        sem = semantics[b, 0]
        msk = mask[b, 0]
        var = variance[b, 0]
        dep = dep_cur[b, 0]
        g1 = pred_log[b, 1]
        hsem = np.full((HH, W), -1, np.int32)
        hmsk = np.zeros((HH, W), np.int32)
        hvar = np.zeros((HH, W), np.float32)
        hdep = np.zeros((HH, W), np.float32)
        hg1 = np.zeros((HH, W), np.float32)
        if r0 - HALO >= 0:
            sl = slice(r0 - HALO, r0)
            hsem[:HALO] = sem[sl]
            hmsk[:HALO] = msk[sl]
            hvar[:HALO] = var[sl]
            hdep[:HALO] = dep[sl]
            hg1[:HALO] = g1[sl]
        if r0 + HB + HALO <= H:
            sl = slice(r0 + HB, r0 + HB + HALO)
            hsem[HB0:HB1] = sem[sl]
            hmsk[HB0:HB1] = msk[sl]
            hvar[HB0:HB1] = var[sl]
            hdep[HB0:HB1] = dep[sl]
            hg1[HB0:HB1] = g1[sl]
        rs = slice(r0, r0 + HB)
        maps.append({
            "sem": np.ascontiguousarray(sem[rs], np.int32),
            "msk": np.ascontiguousarray(msk[rs], np.int32),
            "var": np.ascontiguousarray(var[rs], np.float32),
            "dep": np.ascontiguousarray(dep[rs], np.float32),
            "dpi": np.ascontiguousarray(dep_orig[b, 0][rs], np.float32),
            "g0": np.ascontiguousarray(pred_log[b, 0][rs], np.float32),
            "g1": np.ascontiguousarray(g1[rs], np.float32),
            "hsem": hsem, "hmsk": hmsk, "hvar": hvar, "hdep": hdep, "hg1": hg1,
        })
    return maps


PROFILE = False
LAST_RESULT = None


def _run_once(pred_log, semantics, mask, variance, dep_cur, dep_orig):
    global LAST_RESULT
    from concourse.bass_utils import run_bass_kernel_spmd

    nc = _get_prog()
    in_maps = _core_maps(pred_log, semantics, mask, variance, dep_cur, dep_orig)
    res = run_bass_kernel_spmd(nc, in_maps, core_ids=list(range(8)),
                               trace=PROFILE)
    LAST_RESULT = res
    out = np.empty((BZ, 1, H, W), np.float32)
    for c in range(8):
        b, r = c // 2, c % 2
        out[b, 0, r * HB:(r + 1) * HB] = res.results[c]["out"]
    return out


def kernel(pred_log, semantics, mask, variance, depthin, times=1):
    pred_log = np.asarray(pred_log, np.float32)
    semantics = np.asarray(semantics)
    mask = np.asarray(mask)
    variance = np.asarray(variance, np.float32)
    depthin = np.asarray(depthin, np.float32).reshape(BZ, 1, H, W)
    t = int(np.asarray(times))
    depthout = depthin
    for _ in range(t):
        depthout = _run_once(pred_log, semantics, mask, variance,
                             depthout, depthin)
    if t == 0:
        depthout = depthin.copy()
    return depthout


# revision 21
# speedup vs baseline: 1.9416x; 1.3847x over previous
"""Trainium2 Bass kernel for CRFIntegrationModule.

Math: for each pixel the reference accumulates confidence-weighted depth
estimates from up to 16 same-semantic neighbors in 4 directions (L/R/U/D),
with multiplicative path weights exp(sum of log-gradients), then blends.

Reformulation (validated vs reference in numpy):
  With S = (msk==1 ? sem : -1), Q = (msk==1)*exp(-min(var,5)), q = Q*dep,
  b[n] = [S[n-1]==S[n]], the LEFT-direction accumulators satisfy segmented
  linear recurrences along the row:
    A[n] = b[n]*e^{g[n-1]}*(q[n-1] + A[n-1])      (depth sum, unwindowed)
    B[n] = b[n]*(Q[n-1] + B[n-1])                 (conf sum,  unwindowed)
  and the 16-neighbor window is recovered by subtracting the tail:
    W[n]  = A[n] - gate[n]*e^{C[n-1]-C[n-17]}*A[n-16]
    Wc[n] = B[n] - gate[n]*B[n-16]
  where C = cumsum(g), gate[n] = [NB[n]==NB[n-16]], NB = cumsum([S[n-1]!=S[n]]).
  These map to DVE tensor_tensor_scan ops. RIGHT uses the mirrored recurrence
  (negative-stride scan). U/D are identical along columns, computed in a
  PE-transposed layout (7 column chunks x 168 rows incl. 20-row halos).

Sharding: pure data parallel, 8 cores = 4 images x 2 row-halves (128 rows).
Each core loads its own 20 halo rows; no cross-device communication.
"""
import sys
import numpy as np

sys.path.insert(0, "/opt/trn_rl_repo")

BZ, H, W = 4, 256, 832
HB = 128              # rows per core
PAD = 20              # horizontal pad (window reads reach 17 back)
WP = W + 2 * PAD      # 872
HALO = 20             # vertical halo rows each side
HH = 64               # halo pack: top at partitions 0..19, bottom at 32..51
HB0, HB1 = 32, 52     # bottom-halo partition range (legal matmul base)
CH = HALO + HB + HALO  # 168 rows per transposed chunk
NCH = 7               # 832 = 6*128 + 64 column chunks
VW = NCH * CH         # 1176
CW_FULL = NCH * HB    # 896 (center width in transposed space)
CWS = [128] * 6 + [64]
CLIPVAR = 5.0
LAM = 0.05

_progs = {}


def _build(stage=4, tails=True):
    import concourse.tile as tile
    import concourse.mybir as mybir
    from concourse import bacc, masks
    from contextlib import ExitStack

    Alu = mybir.AluOpType
    Act = mybir.ActivationFunctionType
    f32 = mybir.dt.float32
    i32 = mybir.dt.int32

    nc = bacc.Bacc("TRN2", target_bir_lowering=False, debug=False)

    d_sem = nc.dram_tensor("sem", [HB, W], i32, kind="ExternalInput").ap()
    d_msk = nc.dram_tensor("msk", [HB, W], i32, kind="ExternalInput").ap()
    d_var = nc.dram_tensor("var", [HB, W], f32, kind="ExternalInput").ap()
    d_dep = nc.dram_tensor("dep", [HB, W], f32, kind="ExternalInput").ap()
    d_dpi = nc.dram_tensor("dpi", [HB, W], f32, kind="ExternalInput").ap()
    d_g0 = nc.dram_tensor("g0", [HB, W], f32, kind="ExternalInput").ap()
    d_g1 = nc.dram_tensor("g1", [HB, W], f32, kind="ExternalInput").ap()
    d_hsem = nc.dram_tensor("hsem", [HH, W], i32, kind="ExternalInput").ap()
    d_hmsk = nc.dram_tensor("hmsk", [HH, W], i32, kind="ExternalInput").ap()
    d_hvar = nc.dram_tensor("hvar", [HH, W], f32, kind="ExternalInput").ap()
    d_hdep = nc.dram_tensor("hdep", [HH, W], f32, kind="ExternalInput").ap()
    d_hg1 = nc.dram_tensor("hg1", [HH, W], f32, kind="ExternalInput").ap()
    d_out = nc.dram_tensor("out", [HB, W], f32, kind="ExternalOutput").ap()

    CS = slice(PAD, PAD + W)

    with tile.TileContext(nc) as tc, ExitStack() as ctx:
        pool = ctx.enter_context(tc.tile_pool(name="pool", bufs=1))
        psum = ctx.enter_context(tc.tile_pool(name="psum", bufs=1, space="PSUM"))

        # ---------- loads ----------
        semi = pool.tile([HB, W], i32)
        mski = pool.tile([HB, W], i32)
        var = pool.tile([HB, W], f32)
        dep = pool.tile([HB, W], f32)
        dpi = pool.tile([HB, W], f32)
        g1m = pool.tile([HB, W], f32)
        nc.sync.dma_start(semi[:], d_sem)
        nc.sync.dma_start(mski[:], d_msk)
        nc.sync.dma_start(var[:], d_var)
        nc.sync.dma_start(dep[:], d_dep)
        nc.sync.dma_start(dpi[:], d_dpi)
        nc.sync.dma_start(g1m[:], d_g1)
        g0h = pool.tile([HB, WP], f32)
        nc.gpsimd.memset(g0h[:], 0.0)
        nc.sync.dma_start(g0h[:, CS], d_g0)
        hsemi = pool.tile([HH, W], i32)
        hmski = pool.tile([HH, W], i32)
        hvar = pool.tile([HH, W], f32)
        hdep = pool.tile([HH, W], f32)
        hg1 = pool.tile([HH, W], f32)
        nc.sync.dma_start(hsemi[:], d_hsem)
        nc.sync.dma_start(hmski[:], d_hmsk)
        nc.sync.dma_start(hvar[:], d_hvar)
        nc.sync.dma_start(hdep[:], d_hdep)
        nc.sync.dma_start(hg1[:], d_hg1)

        # ---------- precompute S, Q, q (main + halo) ----------
        S = pool.tile([HB, WP], f32)
        Q = pool.tile([HB, WP], f32)
        q = pool.tile([HB, WP], f32)
        nc.gpsimd.memset(S[:], -1.0)
        nc.gpsimd.memset(Q[:], 0.0)
        nc.gpsimd.memset(q[:], 0.0)

        m1f = pool.tile([HB, W], f32)
        semf = pool.tile([HB, W], f32, tag="pre", bufs=2)
        nc.vector.tensor_copy(semf[:], semi[:])
        nc.vector.tensor_copy(m1f[:], mski[:])
        st = pool.tile([HB, W], f32, tag="pre", bufs=2)
        nc.vector.scalar_tensor_tensor(st[:], semf[:], 1.0, m1f[:],
                                       Alu.add, Alu.mult)
        nc.vector.tensor_scalar_sub(S[:, CS], st[:], 1.0)
        vm = pool.tile([HB, W], f32, tag="pre", bufs=2)
        nc.vector.tensor_scalar(vm[:], var[:], CLIPVAR, -1.0, Alu.min, Alu.mult)
        Etmp = pool.tile([HB, W], f32, tag="pre", bufs=2)
        nc.scalar.activation(Etmp[:], vm[:], Act.Exp)
        nc.vector.tensor_tensor(Q[:, CS], Etmp[:], m1f[:], Alu.mult)
        nc.vector.tensor_tensor(q[:, CS], Q[:, CS], dep[:], Alu.mult)

        hS = pool.tile([HH, W], f32)
        hQ = pool.tile([HH, W], f32)
        hq = pool.tile([HH, W], f32)
        hm1f = pool.tile([HH, W], f32)
        hsemf = pool.tile([HH, W], f32, tag="hpre", bufs=2)
        nc.vector.tensor_copy(hsemf[:], hsemi[:])
        nc.vector.tensor_copy(hm1f[:], hmski[:])
        hst = pool.tile([HH, W], f32, tag="hpre", bufs=2)
        nc.vector.scalar_tensor_tensor(hst[:], hsemf[:], 1.0, hm1f[:],
                                       Alu.add, Alu.mult)
        nc.vector.tensor_scalar_sub(hS[:], hst[:], 1.0)
        hvm = pool.tile([HH, W], f32, tag="hpre", bufs=2)
        nc.vector.tensor_scalar(hvm[:], hvar[:], CLIPVAR, -1.0, Alu.min, Alu.mult)
        hE = pool.tile([HH, W], f32, tag="hpre", bufs=2)
        nc.scalar.activation(hE[:], hvm[:], Act.Exp)
        nc.vector.tensor_tensor(hQ[:], hE[:], hm1f[:], Alu.mult)
        nc.vector.tensor_tensor(hq[:], hQ[:], hdep[:], Alu.mult)

        # ---------- transposed (vertical) planes ----------
        ident = pool.tile([HB, HB], f32)
        masks.make_identity(nc, ident[:])

        Sv = pool.tile([HB, VW], f32)
        Qv = pool.tile([HB, VW], f32)
        qv = pool.tile([HB, VW], f32)
        gv = pool.tile([HB, VW], f32)
        nc.gpsimd.memset(Sv[:], -1.0)
        nc.gpsimd.memset(Qv[:], 0.0)
        nc.gpsimd.memset(qv[:], 0.0)
        nc.gpsimd.memset(gv[:], 0.0)

        plane_srcs = [
            (S, hS, Sv, True),
            (Q, hQ, Qv, True),
            (q, hq, qv, True),
            (g1m, hg1, gv, False),
        ]
        for c in range(NCH if stage >= 2 else 0):
            cw = CWS[c]
            c0 = c * 128
            for (main, halo, dst, padded) in plane_srcs:
                mv = main[:, PAD + c0:PAD + c0 + cw] if padded else main[:, c0:c0 + cw]
                pt1 = psum.tile([HB, HB], f32, tag="ptin1", bufs=2)
                pt2 = psum.tile([HB, HALO], f32, tag="ptin2", bufs=2)
                pt3 = psum.tile([HB, HALO], f32, tag="ptin3", bufs=2)
                nc.tensor.transpose(pt1[:cw, :], mv, ident[:])
                nc.tensor.transpose(pt2[:cw, :], halo[0:HALO, c0:c0 + cw],
                                    ident[0:HALO, 0:HALO])
                nc.tensor.transpose(pt3[:cw, :], halo[HB0:HB1, c0:c0 + cw],
                                    ident[HB0:HB1, HB0:HB1])
                dslot = dst[:cw, c * CH:(c + 1) * CH]
                nc.scalar.copy(dslot[:, HALO:HALO + HB], pt1[:cw, :])
                nc.scalar.copy(dslot[:, 0:HALO], pt2[:cw, :])
                nc.scalar.copy(dslot[:, HALO + HB:CH], pt3[:cw, :])

        # ---------- direction passes ----------
        def directions(Sx, Qx, qx, gx, WD, dsum_add, csum_add):
            """Emit fwd+rev direction pair over [HB, WD] planes. When
            `tails` is False the 16-window tail subtraction is omitted —
            exact whenever no same-semantic run of length >= 16 exists
            (host-checked); the unwindowed scan then equals the window."""
            is_v = WD == VW
            CWIDTH = CW_FULL if is_v else W

            def c3(x):
                return x.rearrange("p (c f) -> p c f", c=NCH) if is_v else x

            def center(x, off):
                if not is_v:
                    return x[:, PAD + off:PAD + off + W]
                v = x.rearrange("p (c f) -> p c f", c=NCH)
                return v[:, :, HALO + off:HALO + off + HB]

            b = pool.tile([HB, VW], f32, tag="dir_b")
            nb = pool.tile([HB, VW], f32, tag="dir_be")
            nc.vector.tensor_tensor(b[:, 1:WD], Sx[:, :WD - 1], Sx[:, 1:WD],
                                    Alu.is_equal)
            nc.vector.memset(b[:, 0:1], 0.0)
            if is_v:
                for c in range(1, NCH):
                    nc.vector.memset(b[:, c * CH:c * CH + 1], 0.0)
            if tails:
                # nb = 1 - b
                nc.vector.tensor_scalar(nb[:, :WD], b[:, :WD], 1.0, -1.0,
                                        Alu.subtract, Alu.mult)
                NBt = pool.tile([HB, VW], f32, tag="dir_NB")
                nc.vector.tensor_tensor_scan(NBt[:, :WD], nb[:, :WD], nb[:, :WD],
                                             0.0, Alu.add, Alu.bypass)
                Ct = pool.tile([HB, VW], f32, tag="dir_C")
                nc.vector.tensor_tensor_scan(Ct[:, :WD], gx[:, :WD], gx[:, :WD],
                                             0.0, Alu.add, Alu.bypass)

            for rev in (False, True):
                eg = pool.tile([HB, VW], f32, tag="dir_eg")
                be = pool.tile([HB, VW], f32, tag="dir_be")
                A = pool.tile([HB, VW], f32, tag="dir_A")
                B = pool.tile([HB, VW], f32, tag="dir_B")
                if not rev:
                    nc.scalar.activation(eg[:, :WD], gx[:, :WD], Act.Exp)
                    nc.vector.tensor_tensor(be[:, 1:WD], b[:, 1:WD],
                                            eg[:, :WD - 1], Alu.mult)
                    nc.vector.memset(A[:, 0:1], 0.0)
                    nc.vector.memset(B[:, 0:1], 0.0)
                    nc.vector.tensor_tensor_scan(
                        A[:, 1:WD], qx[:, :WD - 1], be[:, 1:WD],
                        0.0, Alu.add, Alu.mult)
                    nc.vector.tensor_tensor_scan(
                        B[:, 1:WD], Qx[:, :WD - 1], b[:, 1:WD],
                        0.0, Alu.add, Alu.mult)
                    g_o, t_o = 0, -16          # gate: NB[n] vs NB[n-16]
                    r_a, r_b = -1, -17         # ratio: exp(C[n-1]-C[n-17])
                else:
                    nc.scalar.activation(eg[:, :WD], gx[:, :WD], Act.Exp, scale=-1.0)
                    nc.vector.tensor_tensor(be[:, 0:WD - 1], b[:, 1:WD],
                                            eg[:, :WD - 1], Alu.mult)
                    nc.vector.memset(A[:, WD - 1:WD], 0.0)
                    nc.vector.memset(B[:, WD - 1:WD], 0.0)
                    nc.vector.tensor_tensor_scan(
                        A[:, 0:WD - 1][:, ::-1], qx[:, 1:WD][:, ::-1],
                        be[:, 0:WD - 1][:, ::-1], 0.0, Alu.add, Alu.mult)
                    nc.vector.tensor_tensor_scan(
                        B[:, 0:WD - 1][:, ::-1], Qx[:, 1:WD][:, ::-1],
                        b[:, 1:WD][:, ::-1], 0.0, Alu.add, Alu.mult)
                    g_o, t_o = 16, 16          # gate: NB[n+16] vs NB[n]
                    r_a, r_b = -1, 15          # ratio: exp(C[n-1]-C[n+15])

                dsum_add('+', center(A, 0))
                csum_add('+', center(B, 0))
                if tails:
                    gate = pool.tile([HB, CW_FULL], f32, tag="dir_gate")
                    ratio = pool.tile([HB, CW_FULL], f32, tag="dir_ratio")
                    TA = pool.tile([HB, CW_FULL], f32, tag="dir_TA")
                    TB = pool.tile([HB, CW_FULL], f32, tag="dir_TB")
                    gv_ = c3(gate[:, :CWIDTH])
                    rv_ = c3(ratio[:, :CWIDTH])
                    tv_ = c3(TA[:, :CWIDTH])
                    tb_ = c3(TB[:, :CWIDTH])
                    nc.vector.tensor_tensor(gv_, center(NBt, g_o),
                                            center(NBt, g_o - 16), Alu.is_equal)
                    # ratio = exp(C[a] - C[b]): exact window path sum
                    nc.vector.tensor_tensor(rv_, center(Ct, r_a), center(Ct, r_b),
                                            Alu.subtract)
                    nc.scalar.activation(rv_, rv_, Act.Exp)
                    nc.vector.tensor_tensor(tv_, center(A, t_o), rv_, Alu.mult)
                    nc.vector.tensor_tensor(tv_, tv_, gv_, Alu.mult)
                    dsum_add('-', tv_)
                    nc.vector.tensor_tensor(tb_, center(B, t_o), gv_, Alu.mult)
                    csum_add('-', tb_)

        # ---------- accumulators ----------
        dsum = pool.tile([HB, W], f32)
        csum = pool.tile([HB, W], f32)
        nc.scalar.copy(dsum[:], q[:, CS])
        nc.scalar.copy(csum[:], Q[:, CS])

        def h_acc(acc):
            def add(kind, ap):
                nc.vector.tensor_tensor(
                    acc[:], acc[:], ap, Alu.add if kind == '+' else Alu.subtract)
            return add

        if stage >= 1:
            directions(S, Q, q, g0h, WP, h_acc(dsum), h_acc(csum))

        Wv = pool.tile([HB, CW_FULL], f32)
        Wcv = pool.tile([HB, CW_FULL], f32)
        vstate = {"d": 0, "c": 0}

        def v_acc(acc, key):
            t = acc.rearrange("p (c f) -> p c f", c=NCH)

            def add(kind, ap):
                if vstate[key] == 0:
                    nc.vector.tensor_copy(t[:], ap)
                else:
                    nc.vector.tensor_tensor(
                        t[:], t[:], ap, Alu.add if kind == '+' else Alu.subtract)
                vstate[key] += 1
            return add

        if stage >= 3:
            directions(Sv, Qv, qv, gv, VW, v_acc(Wv, "d"), v_acc(Wcv, "c"))

        for c in range(NCH if stage >= 4 else 0):
            cw = CWS[c]
            c0 = c * 128
            for (src, acc) in ((Wv, dsum), (Wcv, csum)):
                pt = psum.tile([HB, HB], f32, tag="ptout", bufs=2)
                nc.tensor.transpose(pt[:HB, :cw], src[:cw, c * HB:c * HB + HB],
                                    ident[0:cw, 0:cw])
                nc.vector.tensor_tensor(acc[:, c0:c0 + cw], acc[:, c0:c0 + cw],
                                        pt[:HB, :cw], Alu.add)

        # ---------- final blend ----------
        # out = where(lat > 0, LAM*dpi + (1-LAM)*lat, dpi); lat >= 0 always,
        # so "lat nonzero" == "lat > 0" and lat itself serves as the mask.
        dpi05 = pool.tile([HB, W], f32)
        nc.scalar.mul(dpi05[:], dpi[:], LAM)
        outt = pool.tile([HB, W], f32)
        nc.scalar.copy(outt[:], dpi[:])
        mx = pool.tile([HB, W], f32, tag="fin", bufs=3)
        nc.vector.tensor_scalar_max(mx[:], csum[:], 1e-12)
        rcp = pool.tile([HB, W], f32, tag="fin", bufs=3)
        scr = pool.tile([HB, W], f32, tag="fin", bufs=3)
        nc.vector.reciprocal_approx_accurate(rcp[:], mx[:], scr[:])
        lat = pool.tile([HB, W], f32, tag="fin", bufs=3)
        nc.vector.tensor_tensor(lat[:], dsum[:], rcp[:], Alu.mult)
        cp = pool.tile([HB, W], f32, tag="fin", bufs=3)
        nc.vector.scalar_tensor_tensor(cp[:], lat[:], 1.0 - LAM, dpi05[:],
                                       Alu.mult, Alu.add)
        nc.vector.copy_predicated(outt[:], lat[:].bitcast(i32), cp[:])
        nc.sync.dma_start(d_out, outt[:])

    nc.compile()
    return nc


def _get_prog(tails):
    if tails not in _progs:
        _progs[tails] = _build(tails=tails)
    return _progs[tails]


def _needs_tails(semantics, mask):
    """True if any horizontal/vertical run of >=16 equal unmasked semantic
    labels exists — only then can the 16-neighbor window truncate a chain,
    requiring the tail-subtraction program. (Threshold 15 eq-pairs = 16 equal
    pixels, one stricter than the 17 the tail actually needs.)"""
    S = np.where(mask[:, 0] == 1, semantics[:, 0], -1)

    def maxrun(eq):
        # eq: boolean array of consecutive-equal comparisons along last axis
        c = np.cumsum(eq, axis=-1, dtype=np.int32)
        pad = np.zeros((*c.shape[:-1], 1), np.int32)
        c = np.concatenate([pad, c], axis=-1)
        if c.shape[-1] <= 15:
            return False
        return bool((c[..., 15:] - c[..., :-15] == 15).any())

    eq_h = (S[:, :, 1:] == S[:, :, :-1]) & (S[:, :, 1:] >= 0)
    eq_v = (S[:, 1:, :] == S[:, :-1, :]) & (S[:, 1:, :] >= 0)
    return maxrun(eq_h) or maxrun(np.swapaxes(eq_v, 1, 2))


def _core_maps(pred_log, semantics, mask, variance, dep_cur, dep_orig):
    maps = []
    for c in range(8):
        b, r = c // 2, c % 2
        r0 = r * HB
        sem = semantics[b, 0]
        msk = mask[b, 0]
        var = variance[b, 0]
        dep = dep_cur[b, 0]
        g1 = pred_log[b, 1]
        hsem = np.full((HH, W), -1, np.int32)
        hmsk = np.zeros((HH, W), np.int32)
        hvar = np.zeros((HH, W), np.float32)
        hdep = np.zeros((HH, W), np.float32)
        hg1 = np.zeros((HH, W), np.float32)
        if r0 - HALO >= 0:
            sl = slice(r0 - HALO, r0)
            hsem[:HALO] = sem[sl]
            hmsk[:HALO] = msk[sl]
            hvar[:HALO] = var[sl]
            hdep[:HALO] = dep[sl]
            hg1[:HALO] = g1[sl]
        if r0 + HB + HALO <= H:
            sl = slice(r0 + HB, r0 + HB + HALO)
            hsem[HB0:HB1] = sem[sl]
            hmsk[HB0:HB1] = msk[sl]
            hvar[HB0:HB1] = var[sl]
            hdep[HB0:HB1] = dep[sl]
            hg1[HB0:HB1] = g1[sl]
        rs = slice(r0, r0 + HB)
        maps.append({
            "sem": np.ascontiguousarray(sem[rs], np.int32),
            "msk": np.ascontiguousarray(msk[rs], np.int32),
            "var": np.ascontiguousarray(var[rs], np.float32),
            "dep": np.ascontiguousarray(dep[rs], np.float32),
            "dpi": np.ascontiguousarray(dep_orig[b, 0][rs], np.float32),
            "g0": np.ascontiguousarray(pred_log[b, 0][rs], np.float32),
            "g1": np.ascontiguousarray(g1[rs], np.float32),
            "hsem": hsem, "hmsk": hmsk, "hvar": hvar, "hdep": hdep, "hg1": hg1,
        })
    return maps


PROFILE = False
LAST_RESULT = None


def _run_once(pred_log, semantics, mask, variance, dep_cur, dep_orig,
              tails):
    global LAST_RESULT
    from concourse.bass_utils import run_bass_kernel_spmd

    nc = _get_prog(tails)
    in_maps = _core_maps(pred_log, semantics, mask, variance, dep_cur, dep_orig)
    res = run_bass_kernel_spmd(nc, in_maps, core_ids=list(range(8)),
                               trace=PROFILE)
    LAST_RESULT = res
    out = np.empty((BZ, 1, H, W), np.float32)
    for c in range(8):
        b, r = c // 2, c % 2
        out[b, 0, r * HB:(r + 1) * HB] = res.results[c]["out"]
    return out


def kernel(pred_log, semantics, mask, variance, depthin, times=1):
    pred_log = np.asarray(pred_log, np.float32)
    semantics = np.asarray(semantics)
    mask = np.asarray(mask)
    variance = np.asarray(variance, np.float32)
    depthin = np.asarray(depthin, np.float32).reshape(BZ, 1, H, W)
    t = int(np.asarray(times))
    tails = _needs_tails(semantics, mask)
    depthout = depthin
    for _ in range(t):
        depthout = _run_once(pred_log, semantics, mask, variance,
                             depthout, depthin, tails)
    if t == 0:
        depthout = depthin.copy()
    return depthout


# revision 22
# speedup vs baseline: 2.1611x; 1.1131x over previous
"""Trainium2 Bass kernel for CRFIntegrationModule.

Math: for each pixel the reference accumulates confidence-weighted depth
estimates from up to 16 same-semantic neighbors in 4 directions (L/R/U/D),
with multiplicative path weights exp(sum of log-gradients), then blends.

Reformulation (validated vs reference in numpy):
  With S = (msk==1 ? sem : -1), Q = (msk==1)*exp(-min(var,5)), q = Q*dep,
  b[n] = [S[n-1]==S[n]], the LEFT-direction accumulators satisfy segmented
  linear recurrences along the row:
    A[n] = b[n]*e^{g[n-1]}*(q[n-1] + A[n-1])      (depth sum, unwindowed)
    B[n] = b[n]*(Q[n-1] + B[n-1])                 (conf sum,  unwindowed)
  and the 16-neighbor window is recovered by subtracting the tail:
    W[n]  = A[n] - gate[n]*e^{C[n-1]-C[n-17]}*A[n-16]
    Wc[n] = B[n] - gate[n]*B[n-16]
  where C = cumsum(g), gate[n] = [NB[n]==NB[n-16]], NB = cumsum([S[n-1]!=S[n]]).
  These map to DVE tensor_tensor_scan ops. RIGHT uses the mirrored recurrence
  (negative-stride scan). U/D are identical along columns, computed in a
  PE-transposed layout (7 column chunks x 168 rows incl. 20-row halos).

The tail terms are nonzero only where >=17 consecutive pixels share one
unmasked semantic label. The host checks the actual input for such runs
(threshold 16, one stricter) and dispatches a fast program without the tail
machinery when none exist — bit-exact for that input — falling back to the
full program otherwise. Halo strips arrive from the host already transposed
(pure layout), so only the 128-row main blocks go through PE transposes.

Sharding: pure data parallel, 8 cores = 4 images x 2 row-halves (128 rows).
Each core loads its own 20 halo rows; no cross-device communication.
"""
import sys
import numpy as np

sys.path.insert(0, "/opt/trn_rl_repo")

BZ, H, W = 4, 256, 832
HB = 128              # rows per core
PAD = 20              # horizontal pad (window reads reach 17 back)
WP = W + 2 * PAD      # 872
HALO = 20             # vertical halo rows each side
HW2 = 2 * HALO        # 40 halo rows total per column
CH = HALO + HB + HALO  # 168 rows per transposed chunk
NCH = 7               # 832 = 6*128 + 64 column chunks
VW = NCH * CH         # 1176
CW_FULL = NCH * HB    # 896 (center width in transposed space)
HTW = NCH * HW2       # 280 (packed transposed-halo width)
CWS = [128] * 6 + [64]
CLIPVAR = 5.0
LAM = 0.05

_progs = {}


def _build(tails=True):
    import concourse.tile as tile
    import concourse.mybir as mybir
    from concourse import bacc, masks
    from contextlib import ExitStack

    Alu = mybir.AluOpType
    Act = mybir.ActivationFunctionType
    f32 = mybir.dt.float32
    i32 = mybir.dt.int32

    nc = bacc.Bacc("TRN2", target_bir_lowering=False, debug=False)

    d_sem = nc.dram_tensor("sem", [HB, W], i32, kind="ExternalInput").ap()
    d_msk = nc.dram_tensor("msk", [HB, W], i32, kind="ExternalInput").ap()
    d_var = nc.dram_tensor("var", [HB, W], f32, kind="ExternalInput").ap()
    d_dep = nc.dram_tensor("dep", [HB, W], f32, kind="ExternalInput").ap()
    d_dpi = nc.dram_tensor("dpi", [HB, W], f32, kind="ExternalInput").ap()
    d_g0 = nc.dram_tensor("g0", [HB, W], f32, kind="ExternalInput").ap()
    d_g1 = nc.dram_tensor("g1", [HB, W], f32, kind="ExternalInput").ap()
    # transposed halo planes, host-packed [128, NCH*40]:
    # [p, c*40+j] = plane[halo_row_j, col c*128+p]  (j<20 top, j>=20 bottom)
    d_hsem = nc.dram_tensor("hsem", [HB, HTW], i32, kind="ExternalInput").ap()
    d_hmsk = nc.dram_tensor("hmsk", [HB, HTW], i32, kind="ExternalInput").ap()
    d_hvar = nc.dram_tensor("hvar", [HB, HTW], f32, kind="ExternalInput").ap()
    d_hdep = nc.dram_tensor("hdep", [HB, HTW], f32, kind="ExternalInput").ap()
    d_hg1 = nc.dram_tensor("hg1", [HB, HTW], f32, kind="ExternalInput").ap()
    d_out = nc.dram_tensor("out", [HB, W], f32, kind="ExternalOutput").ap()

    CS = slice(PAD, PAD + W)

    with tile.TileContext(nc) as tc, ExitStack() as ctx:
        pool = ctx.enter_context(tc.tile_pool(name="pool", bufs=1))
        psum = ctx.enter_context(tc.tile_pool(name="psum", bufs=1, space="PSUM"))

        # ---------- loads ----------
        semi = pool.tile([HB, W], i32)
        mski = pool.tile([HB, W], i32)
        var = pool.tile([HB, W], f32)
        dep = pool.tile([HB, W], f32)
        dpi = pool.tile([HB, W], f32)
        g1m = pool.tile([HB, W], f32)
        nc.sync.dma_start(semi[:], d_sem)
        nc.sync.dma_start(mski[:], d_msk)
        nc.sync.dma_start(var[:], d_var)
        nc.sync.dma_start(dep[:], d_dep)
        nc.sync.dma_start(dpi[:], d_dpi)
        nc.sync.dma_start(g1m[:], d_g1)
        g0h = pool.tile([HB, WP], f32)
        nc.gpsimd.memset(g0h[:], 0.0)
        nc.sync.dma_start(g0h[:, CS], d_g0)
        hsemT = pool.tile([HB, HTW], i32)
        hmskT = pool.tile([HB, HTW], i32)
        hvarT = pool.tile([HB, HTW], f32)
        hdepT = pool.tile([HB, HTW], f32)
        nc.sync.dma_start(hsemT[:], d_hsem)
        nc.sync.dma_start(hmskT[:], d_hmsk)
        nc.sync.dma_start(hvarT[:], d_hvar)
        nc.sync.dma_start(hdepT[:], d_hdep)

        # ---------- main precompute S, Q, q ----------
        S = pool.tile([HB, WP], f32)
        Q = pool.tile([HB, WP], f32)
        q = pool.tile([HB, WP], f32)
        nc.gpsimd.memset(S[:], -1.0)
        nc.gpsimd.memset(Q[:], 0.0)
        nc.gpsimd.memset(q[:], 0.0)

        m1f = pool.tile([HB, W], f32)
        semf = pool.tile([HB, W], f32, tag="pre", bufs=2)
        nc.scalar.copy(semf[:], semi[:])
        nc.scalar.copy(m1f[:], mski[:])
        st = pool.tile([HB, W], f32, tag="pre", bufs=2)
        nc.vector.scalar_tensor_tensor(st[:], semf[:], 1.0, m1f[:],
                                       Alu.add, Alu.mult)
        nc.vector.tensor_scalar_sub(S[:, CS], st[:], 1.0)
        vm = pool.tile([HB, W], f32, tag="pre", bufs=2)
        nc.vector.tensor_scalar(vm[:], var[:], CLIPVAR, -1.0, Alu.min, Alu.mult)
        Etmp = pool.tile([HB, W], f32, tag="pre", bufs=2)
        nc.scalar.activation(Etmp[:], vm[:], Act.Exp)
        nc.vector.tensor_tensor(Q[:, CS], Etmp[:], m1f[:], Alu.mult)
        nc.vector.tensor_tensor(q[:, CS], Q[:, CS], dep[:], Alu.mult)

        # ---------- transposed (vertical) planes ----------
        ident = pool.tile([HB, HB], f32)
        masks.make_identity(nc, ident[:])

        Sv = pool.tile([HB, VW], f32)
        Qv = pool.tile([HB, VW], f32)
        qv = pool.tile([HB, VW], f32)
        gv = pool.tile([HB, VW], f32)
        nc.gpsimd.memset(Sv[:], -1.0)
        nc.gpsimd.memset(Qv[:], 0.0)
        nc.gpsimd.memset(qv[:], 0.0)
        nc.gpsimd.memset(gv[:], 0.0)

        def strips(x):
            v = x.rearrange("p (c f) -> p c f", c=NCH)
            return v[:, :, 0:HALO], v[:, :, HALO + HB:CH]

        def hv(x, half):
            v = x.rearrange("p (c j) -> p c j", c=NCH)
            return v[:, :, 0:HALO] if half == 0 else v[:, :, HALO:HW2]

        # halo precompute directly into the V-plane halo strips
        hsemfT = pool.tile([HB, HTW], f32, tag="hpre", bufs=2)
        hm1fT = pool.tile([HB, HTW], f32)
        nc.scalar.copy(hsemfT[:], hsemT[:])
        nc.scalar.copy(hm1fT[:], hmskT[:])
        sth = pool.tile([HB, HTW], f32, tag="hpre", bufs=2)
        nc.vector.scalar_tensor_tensor(sth[:], hsemfT[:], 1.0, hm1fT[:],
                                       Alu.add, Alu.mult)
        Sv_t, Sv_b = strips(Sv)
        nc.vector.tensor_scalar_sub(Sv_t, hv(sth, 0), 1.0)
        nc.vector.tensor_scalar_sub(Sv_b, hv(sth, 1), 1.0)
        vmh = pool.tile([HB, HTW], f32, tag="hpre", bufs=2)
        nc.vector.tensor_scalar(vmh[:], hvarT[:], CLIPVAR, -1.0, Alu.min, Alu.mult)
        EhT = pool.tile([HB, HTW], f32, tag="hpre", bufs=2)
        nc.scalar.activation(EhT[:], vmh[:], Act.Exp)
        Qv_t, Qv_b = strips(Qv)
        nc.vector.tensor_tensor(Qv_t, hv(EhT, 0), hv(hm1fT, 0), Alu.mult)
        nc.vector.tensor_tensor(Qv_b, hv(EhT, 1), hv(hm1fT, 1), Alu.mult)
        qv_t, qv_b = strips(qv)
        nc.vector.tensor_tensor(qv_t, Qv_t, hv(hdepT, 0), Alu.mult)
        nc.vector.tensor_tensor(qv_b, Qv_b, hv(hdepT, 1), Alu.mult)
        gv_t, gv_b = strips(gv)
        hg3 = d_hg1.rearrange("p (c j) -> p c j", c=NCH)
        nc.sync.dma_start(gv_t, hg3[:, :, 0:HALO])
        nc.sync.dma_start(gv_b, hg3[:, :, HALO:HW2])

        # main-block PE transposes into chunk interiors
        plane_srcs = [(S, Sv, True), (Q, Qv, True), (q, qv, True),
                      (g1m, gv, False)]
        for c in range(NCH):
            cw = CWS[c]
            c0 = c * 128
            for (main, dst, padded) in plane_srcs:
                mv = main[:, PAD + c0:PAD + c0 + cw] if padded else main[:, c0:c0 + cw]
                pt1 = psum.tile([HB, HB], f32, tag="ptin1", bufs=4)
                nc.tensor.transpose(pt1[:cw, :], mv, ident[:])
                nc.scalar.copy(dst[:cw, c * CH + HALO:c * CH + HALO + HB],
                               pt1[:cw, :])

        # ---------- direction passes ----------
        def directions(Sx, Qx, qx, gx, WD, dsum_add, csum_add):
            """Emit fwd+rev direction pair over [HB, WD] planes. When
            `tails` is False the 16-window tail subtraction is omitted —
            exact whenever no same-semantic run of >=16 exists (host-checked):
            the unwindowed scan then equals the windowed sum."""
            is_v = WD == VW
            CWIDTH = CW_FULL if is_v else W

            def c3(x):
                return x.rearrange("p (c f) -> p c f", c=NCH) if is_v else x

            def center(x, off):
                if not is_v:
                    return x[:, PAD + off:PAD + off + W]
                v = x.rearrange("p (c f) -> p c f", c=NCH)
                return v[:, :, HALO + off:HALO + off + HB]

            b = pool.tile([HB, VW], f32, tag="dir_b")
            nb = pool.tile([HB, VW], f32, tag="dir_be")
            nc.vector.tensor_tensor(b[:, 1:WD], Sx[:, :WD - 1], Sx[:, 1:WD],
                                    Alu.is_equal)
            nc.vector.memset(b[:, 0:1], 0.0)
            if is_v:
                for c in range(1, NCH):
                    nc.vector.memset(b[:, c * CH:c * CH + 1], 0.0)
            if tails:
                nc.vector.tensor_scalar(nb[:, :WD], b[:, :WD], 1.0, -1.0,
                                        Alu.subtract, Alu.mult)
                NBt = pool.tile([HB, VW], f32, tag="dir_NB")
                nc.vector.tensor_tensor_scan(NBt[:, :WD], nb[:, :WD], nb[:, :WD],
                                             0.0, Alu.add, Alu.bypass)
                Ct = pool.tile([HB, VW], f32, tag="dir_C")
                nc.vector.tensor_tensor_scan(Ct[:, :WD], gx[:, :WD], gx[:, :WD],
                                             0.0, Alu.add, Alu.bypass)

            for rev in (False, True):
                eg = pool.tile([HB, VW], f32, tag="dir_eg")
                be = pool.tile([HB, VW], f32, tag="dir_be")
                A = pool.tile([HB, VW], f32, tag="dir_A", bufs=2)
                B = pool.tile([HB, VW], f32, tag="dir_B", bufs=2)
                if not rev:
                    nc.scalar.activation(eg[:, :WD], gx[:, :WD], Act.Exp)
                    nc.vector.tensor_tensor(be[:, 1:WD], b[:, 1:WD],
                                            eg[:, :WD - 1], Alu.mult)
                    nc.vector.memset(A[:, 0:1], 0.0)
                    nc.vector.memset(B[:, 0:1], 0.0)
                    nc.vector.tensor_tensor_scan(
                        A[:, 1:WD], qx[:, :WD - 1], be[:, 1:WD],
                        0.0, Alu.add, Alu.mult)
                    nc.vector.tensor_tensor_scan(
                        B[:, 1:WD], Qx[:, :WD - 1], b[:, 1:WD],
                        0.0, Alu.add, Alu.mult)
                    g_o, t_o = 0, -16          # gate: NB[n] vs NB[n-16]
                    r_a, r_b = -1, -17         # ratio: exp(C[n-1]-C[n-17])
                else:
                    nc.scalar.activation(eg[:, :WD], gx[:, :WD], Act.Exp, scale=-1.0)
                    nc.vector.tensor_tensor(be[:, 0:WD - 1], b[:, 1:WD],
                                            eg[:, :WD - 1], Alu.mult)
                    nc.vector.memset(A[:, WD - 1:WD], 0.0)
                    nc.vector.memset(B[:, WD - 1:WD], 0.0)
                    nc.vector.tensor_tensor_scan(
                        A[:, 0:WD - 1][:, ::-1], qx[:, 1:WD][:, ::-1],
                        be[:, 0:WD - 1][:, ::-1], 0.0, Alu.add, Alu.mult)
                    nc.vector.tensor_tensor_scan(
                        B[:, 0:WD - 1][:, ::-1], Qx[:, 1:WD][:, ::-1],
                        b[:, 1:WD][:, ::-1], 0.0, Alu.add, Alu.mult)
                    g_o, t_o = 16, 16          # gate: NB[n+16] vs NB[n]
                    r_a, r_b = -1, 15          # ratio: exp(C[n-1]-C[n+15])

                dsum_add('+', center(A, 0), A)
                csum_add('+', center(B, 0), B)
                if tails:
                    gate = pool.tile([HB, CW_FULL], f32, tag="dir_gate")
                    ratio = pool.tile([HB, CW_FULL], f32, tag="dir_ratio")
                    TA = pool.tile([HB, CW_FULL], f32, tag="dir_TA")
                    TB = pool.tile([HB, CW_FULL], f32, tag="dir_TB")
                    gv_ = c3(gate[:, :CWIDTH])
                    rv_ = c3(ratio[:, :CWIDTH])
                    tv_ = c3(TA[:, :CWIDTH])
                    tb_ = c3(TB[:, :CWIDTH])
                    nc.vector.tensor_tensor(gv_, center(NBt, g_o),
                                            center(NBt, g_o - 16), Alu.is_equal)
                    nc.vector.tensor_tensor(rv_, center(Ct, r_a), center(Ct, r_b),
                                            Alu.subtract)
                    nc.scalar.activation(rv_, rv_, Act.Exp)
                    nc.vector.tensor_tensor(tv_, center(A, t_o), rv_, Alu.mult)
                    nc.vector.tensor_tensor(tv_, tv_, gv_, Alu.mult)
                    dsum_add('-', tv_, None)
                    nc.vector.tensor_tensor(tb_, center(B, t_o), gv_, Alu.mult)
                    csum_add('-', tb_, None)

        # ---------- accumulators ----------
        dsum = pool.tile([HB, W], f32)
        csum = pool.tile([HB, W], f32)
        nc.scalar.copy(dsum[:], q[:, CS])
        nc.scalar.copy(csum[:], Q[:, CS])

        def h_acc(acc):
            def add(kind, ap, _tile=None):
                nc.vector.tensor_tensor(
                    acc[:], acc[:], ap, Alu.add if kind == '+' else Alu.subtract)
            return add

        directions(S, Q, q, g0h, WP, h_acc(dsum), h_acc(csum))

        if not tails:
            # fast path: no Wv/Wcv accumulation — collect the V-pass A/B tiles
            # and PSUM-accumulate their per-chunk back-transposes directly.
            coll = {"d": [], "c": []}

            def v_acc(key):
                def add(kind, ap, tile_):
                    assert kind == '+' and tile_ is not None
                    coll[key].append(tile_)
                return add

            directions(Sv, Qv, qv, gv, VW, v_acc("d"), v_acc("c"))
            for c in range(NCH):
                cw = CWS[c]
                c0 = c * 128
                csl = slice(c * CH + HALO, c * CH + HALO + HB)
                for key, acc in (("d", dsum), ("c", csum)):
                    pt = psum.tile([HB, HB], f32, tag="ptout", bufs=4)
                    t0, t1 = coll[key]
                    nc.tensor.matmul(pt[:HB, :cw], t0[:cw, csl],
                                     ident[0:cw, 0:cw],
                                     is_transpose=True, start=True, stop=False)
                    nc.tensor.matmul(pt[:HB, :cw], t1[:cw, csl],
                                     ident[0:cw, 0:cw],
                                     is_transpose=True, start=False, stop=True)
                    nc.vector.tensor_tensor(acc[:, c0:c0 + cw],
                                            acc[:, c0:c0 + cw],
                                            pt[:HB, :cw], Alu.add)
        else:
            Wv = pool.tile([HB, CW_FULL], f32)
            Wcv = pool.tile([HB, CW_FULL], f32)
            vstate = {"d": 0, "c": 0}

            def v_accw(acc, key):
                t = acc.rearrange("p (c f) -> p c f", c=NCH)

                def add(kind, ap, _tile=None):
                    if vstate[key] == 0:
                        nc.vector.tensor_copy(t[:], ap)
                    else:
                        nc.vector.tensor_tensor(
                            t[:], t[:], ap,
                            Alu.add if kind == '+' else Alu.subtract)
                    vstate[key] += 1
                return add

            directions(Sv, Qv, qv, gv, VW, v_accw(Wv, "d"), v_accw(Wcv, "c"))
            for c in range(NCH):
                cw = CWS[c]
                c0 = c * 128
                for (src, acc) in ((Wv, dsum), (Wcv, csum)):
                    pt = psum.tile([HB, HB], f32, tag="ptout", bufs=4)
                    nc.tensor.transpose(pt[:HB, :cw],
                                        src[:cw, c * HB:c * HB + HB],
                                        ident[0:cw, 0:cw])
                    nc.vector.tensor_tensor(acc[:, c0:c0 + cw],
                                            acc[:, c0:c0 + cw],
                                            pt[:HB, :cw], Alu.add)

        # ---------- final blend ----------
        # out = where(lat > 0, LAM*dpi + (1-LAM)*lat, dpi); lat >= 0 always,
        # so "lat nonzero" == "lat > 0" and lat's bits serve as the mask.
        dpi05 = pool.tile([HB, W], f32)
        nc.scalar.mul(dpi05[:], dpi[:], LAM)
        outt = pool.tile([HB, W], f32)
        nc.scalar.copy(outt[:], dpi[:])
        mx = pool.tile([HB, W], f32, tag="fin", bufs=3)
        nc.vector.tensor_scalar_max(mx[:], csum[:], 1e-12)
        rcp = pool.tile([HB, W], f32, tag="fin", bufs=3)
        scr = pool.tile([HB, W], f32, tag="fin", bufs=3)
        nc.vector.reciprocal_approx_accurate(rcp[:], mx[:], scr[:])
        lat = pool.tile([HB, W], f32, tag="fin", bufs=3)
        nc.vector.tensor_tensor(lat[:], dsum[:], rcp[:], Alu.mult)
        cp = pool.tile([HB, W], f32, tag="fin", bufs=3)
        nc.vector.scalar_tensor_tensor(cp[:], lat[:], 1.0 - LAM, dpi05[:],
                                       Alu.mult, Alu.add)
        nc.vector.copy_predicated(outt[:], lat[:].bitcast(i32), cp[:])
        nc.sync.dma_start(d_out, outt[:])

    nc.compile()
    return nc


def _get_prog(tails):
    if tails not in _progs:
        _progs[tails] = _build(tails=tails)
    return _progs[tails]


def _needs_tails(semantics, mask):
    """True if any horizontal/vertical run of >=16 equal unmasked semantic
    labels exists — only then can the 16-neighbor window truncate a chain,
    requiring the tail-subtraction program. (Threshold 15 eq-pairs = 16 equal
    pixels, one stricter than the 17 the tail actually needs.)"""
    S = np.where(mask[:, 0] == 1, semantics[:, 0], -1)

    def maxrun(eq):
        c = np.cumsum(eq, axis=-1, dtype=np.int32)
        pad = np.zeros((*c.shape[:-1], 1), np.int32)
        c = np.concatenate([pad, c], axis=-1)
        if c.shape[-1] <= 15:
            return False
        return bool((c[..., 15:] - c[..., :-15] == 15).any())

    eq_h = (S[:, :, 1:] == S[:, :, :-1]) & (S[:, :, 1:] >= 0)
    eq_v = (S[:, 1:, :] == S[:, :-1, :]) & (S[:, 1:, :] >= 0)
    return maxrun(eq_h) or maxrun(np.swapaxes(eq_v, 1, 2))


def _pack_halo_T(plane, r0, fill, dtype):
    """Pack the 40 halo rows (20 above r0, 20 below r0+HB; out-of-image ->
    fill) into the transposed layout [128, NCH*40]."""
    out = np.full((HB, HTW), fill, dtype)
    halo = np.full((HW2, W), fill, dtype)
    if r0 - HALO >= 0:
        halo[:HALO] = plane[r0 - HALO:r0]
    if r0 + HB + HALO <= H:
        halo[HALO:] = plane[r0 + HB:r0 + HB + HALO]
    for c in range(NCH):
        cw = CWS[c]
        out[:cw, c * HW2:(c + 1) * HW2] = halo[:, c * 128:c * 128 + cw].T
    return out


def _core_maps(pred_log, semantics, mask, variance, dep_cur, dep_orig):
    maps = []
    for c in range(8):
        b, r = c // 2, c % 2
        r0 = r * HB
        sem = semantics[b, 0]
        msk = mask[b, 0]
        var = variance[b, 0]
        dep = dep_cur[b, 0]
        g1 = pred_log[b, 1]
        rs = slice(r0, r0 + HB)
        maps.append({
            "sem": np.ascontiguousarray(sem[rs], np.int32),
            "msk": np.ascontiguousarray(msk[rs], np.int32),
            "var": np.ascontiguousarray(var[rs], np.float32),
            "dep": np.ascontiguousarray(dep[rs], np.float32),
            "dpi": np.ascontiguousarray(dep_orig[b, 0][rs], np.float32),
            "g0": np.ascontiguousarray(pred_log[b, 0][rs], np.float32),
            "g1": np.ascontiguousarray(g1[rs], np.float32),
            "hsem": _pack_halo_T(sem, r0, -1, np.int32),
            "hmsk": _pack_halo_T(msk, r0, 0, np.int32),
            "hvar": _pack_halo_T(var, r0, 0.0, np.float32),
            "hdep": _pack_halo_T(dep, r0, 0.0, np.float32),
            "hg1": _pack_halo_T(g1, r0, 0.0, np.float32),
        })
    return maps


PROFILE = False
LAST_RESULT = None


def _run_once(pred_log, semantics, mask, variance, dep_cur, dep_orig, tails):
    global LAST_RESULT
    from concourse.bass_utils import run_bass_kernel_spmd

    nc = _get_prog(tails)
    in_maps = _core_maps(pred_log, semantics, mask, variance, dep_cur, dep_orig)
    res = run_bass_kernel_spmd(nc, in_maps, core_ids=list(range(8)),
                               trace=PROFILE)
    LAST_RESULT = res
    out = np.empty((BZ, 1, H, W), np.float32)
    for c in range(8):
        b, r = c // 2, c % 2
        out[b, 0, r * HB:(r + 1) * HB] = res.results[c]["out"]
    return out


def kernel(pred_log, semantics, mask, variance, depthin, times=1):
    pred_log = np.asarray(pred_log, np.float32)
    semantics = np.asarray(semantics)
    mask = np.asarray(mask)
    variance = np.asarray(variance, np.float32)
    depthin = np.asarray(depthin, np.float32).reshape(BZ, 1, H, W)
    t = int(np.asarray(times))
    tails = _needs_tails(semantics, mask)
    depthout = depthin
    for _ in range(t):
        depthout = _run_once(pred_log, semantics, mask, variance,
                             depthout, depthin, tails)
    if t == 0:
        depthout = depthin.copy()
    return depthout
